# revision 33
# baseline (speedup 1.0000x reference)
"""AFNO2D (DHT-spectral block-MLP) Trainium2 kernel, 8-core SPMD.

Pipeline (per core):
  stage A (H-shard, 16 rows):  W-DFT 256->65 (trunc) then C-DFT 768, as matmuls
  AllToAll (split per batch, overlapped with stage A) -> C-shard
  stage B: fused (B,H)-DFT + Re+Im -> a ; block MLP + softshrink -> s ;
           (B,H)-DFT of s -> (re3,im3)
  AllToAll (split by w' halves, overlapped with stage B) -> H-shard
  stage C: C-IDFT (3-product Karatsuba), W-expand 65->256 + 1/numel, + x bias
All heavy compute is TensorE matmuls vs host-precomputed cos/sin DFT matrices.

Layouts: forward a2a chunk = [h][r][c][w] per batch (h-outer; stage-B reads
contiguous per h-row).  Backward a2a chunk = [b][r][c][h][w-half] (c-outer;
stage-B writes and stage-C reads get 1-4KB contiguous runs per partition).

Stage C Karatsuba: with A=cc.rcr, B=sc.rci, C2=cc.rci, D=sc.rcr:
  k1 = cc.(rcr+rci) = A+C2 ; k2 = -(cc+sc).rcr = -A-D ; k3 = (cc-sc).rci = C2-B
  re4 = k1-k3 = A+B ; im4 = k1+k2 = C2-D   (18 matmuls instead of 24)
"""
import math
import numpy as np
import ml_dtypes

import concourse.bass as bass
from concourse import bacc
import concourse.mybir as mybir
from concourse import tile
from concourse.bass_utils import run_bass_kernel_spmd

F32 = mybir.dt.float32
F32R = mybir.dt.float32r
BF16 = mybir.dt.bfloat16

B, H, W, C = 2, 128, 256, 768
NCORES = 8
HS = H // NCORES          # 16
KEPT = 65
BS = 96                   # block size (channels per core in stage B)
NB = 8
WSP = 32                  # backward a2a w' split point
CHUNKF = HS * 2 * BS * KEPT          # fwd chunk per batch: [h][r][c][w]
CHUNKB0 = B * 2 * BS * HS * WSP      # bwd chunk, w' 0:32
CHUNKB1 = B * 2 * BS * HS * (KEPT - WSP)   # bwd chunk, w' 32:65
ACT_T = mybir.ActivationFunctionType
ALU = mybir.AluOpType


def _cs(n, m=None):
    i = np.arange(n)[:, None].astype(np.float64)
    k = np.arange(m if m is not None else n)[None, :].astype(np.float64)
    th = 2.0 * np.pi * i * k / n
    return np.cos(th), np.sin(th)


def _host_consts():
    CW, SW = _cs(W, KEPT)
    CC, SC = _cs(C)
    CH, SH = _cs(H)
    F2 = np.array([[1.0, 1.0], [1.0, -1.0]])
    KBHc = np.kron(F2, CH)
    KBHs = np.kron(F2, SH)
    Mm = KBHc - KBHs
    Mp = KBHc + KBHs
    CW2, SW2 = _cs(W)
    numel = float(B * H * W * C)
    Wm = ((CW2 - SW2)[:KEPT] / numel)
    Wp = ((CW2 + SW2)[:KEPT] / numel)

    dw = np.zeros((W, 130), np.float64)
    dw[:, :KEPT] = CW
    dw[:, KEPT:130] = -SW

    def bf(a):
        return np.ascontiguousarray(a).astype(ml_dtypes.bfloat16)

    def kmaj(a, nk):          # (nk*128, F) -> (128, nk*F) k-tiles along free axis
        nkF = a.shape[1]
        return np.ascontiguousarray(
            a.reshape(nk, 128, nkF).transpose(1, 0, 2).reshape(128, nk * nkF))

    consts = {
        "dw": bf(kmaj(dw, 2)),
        "cc": bf(kmaj(CC, 6)),
        "sc": bf(kmaj(SC, 6)),
        "scn": bf(kmaj(-SC, 6)),
        "kpc": bf(kmaj(-(CC + SC), 6)),
        "kmc": bf(kmaj(CC - SC, 6)),
        "mm": bf(kmaj(Mm, 2)),
        "mp": bf(kmaj(Mp, 2)),
        "kc": bf(kmaj(KBHc, 2)),
        "ksn": bf(kmaj(-KBHs, 2)),
        "wm": bf(Wm.reshape(KEPT, 256)),
        "wp": bf(Wp.reshape(KEPT, 256)),
        "ident": bf(np.eye(128)),
        "identf": np.eye(128, dtype=np.float32),
    }
    return consts


# w'-groups for stage B: (start, count); 65 = 16*4 + 1
GROUPS = [(i * 4, 4) for i in range(16)] + [(64, 1)]
USE_COLLECTIVES = True

# stage C: c-slab [128k, 128k+128) split at 96-block boundaries -> (j, lo, hi)
def _slab_splits(k):
    out = []
    c_lo, c_hi = 128 * k, 128 * (k + 1)
    j = c_lo // BS
    while BS * j < c_hi:
        lo = max(BS * j, c_lo)
        hi = min(BS * j + BS, c_hi)
        if hi > lo:
            out.append((j, lo, hi))
        j += 1
    return out


def build_nc():
    nc = bacc.Bacc()
    x = nc.declare_dram_parameter("x", [B, HS, W, C], F32, isOutput=False)
    an = nc.declare_dram_parameter("an", [BS, KEPT * 256], BF16, isOutput=False)
    dw = nc.declare_dram_parameter("dw", [128, 2 * 130], BF16, isOutput=False)
    cc = nc.declare_dram_parameter("cc", [128, 6 * C], BF16, isOutput=False)
    sc = nc.declare_dram_parameter("sc", [128, 6 * C], BF16, isOutput=False)
    scn = nc.declare_dram_parameter("scn", [128, 6 * C], BF16, isOutput=False)
    kpc = nc.declare_dram_parameter("kpc", [128, 6 * C], BF16, isOutput=False)
    kmc = nc.declare_dram_parameter("kmc", [128, 6 * C], BF16, isOutput=False)
    mmp = nc.declare_dram_parameter("mm", [128, 2 * 256], BF16, isOutput=False)
    mpp = nc.declare_dram_parameter("mp", [128, 2 * 256], BF16, isOutput=False)
    kcp = nc.declare_dram_parameter("kc", [128, 2 * 256], BF16, isOutput=False)
    ksnp = nc.declare_dram_parameter("ksn", [128, 2 * 256], BF16, isOutput=False)
    wmp = nc.declare_dram_parameter("wm", [KEPT, 2 * 128], BF16, isOutput=False)
    wpp = nc.declare_dram_parameter("wp", [KEPT, 2 * 128], BF16, isOutput=False)
    identp = nc.declare_dram_parameter("ident", [128, 128], BF16, isOutput=False)
    identfp = nc.declare_dram_parameter("identf", [128, 128], F32, isOutput=False)
    w1p = nc.declare_dram_parameter("w1p", [BS, BS], BF16, isOutput=False)
    w1m = nc.declare_dram_parameter("w1m", [BS, BS], BF16, isOutput=False)
    w2p = nc.declare_dram_parameter("w2p", [BS, BS], BF16, isOutput=False)
    w2m = nc.declare_dram_parameter("w2m", [BS, BS], BF16, isOutput=False)
    b10 = nc.declare_dram_parameter("b10", [BS, 1], F32, isOutput=False)
    b11 = nc.declare_dram_parameter("b11", [BS, 1], F32, isOutput=False)
    b20 = nc.declare_dram_parameter("b20", [BS, 1], F32, isOutput=False)
    b21 = nc.declare_dram_parameter("b21", [BS, 1], F32, isOutput=False)
    bsv = nc.declare_dram_parameter("bsv", [BS, 1], F32, isOutput=False)
    out = nc.declare_dram_parameter("out", [B, HS, W, C], F32, isOutput=True)

    CHUNK = B * CHUNKF
    a2a_in = nc.dram_tensor("a2a_in", [NCORES, CHUNK], BF16)
    a2a_out = nc.dram_tensor("a2a_out", [NCORES, CHUNK], BF16)
    bk_in = nc.dram_tensor("bk_in", [NCORES, CHUNK], BF16)
    bk_out = nc.dram_tensor("bk_out", [NCORES, CHUNK], BF16)
    RG = [list(range(NCORES))]

    # DRAM views.  forward: chunk flat order (b, h, r, c, w)
    av_send = a2a_in.rearrange("j (b h r c w) -> j b r c h w", b=B, h=HS, r=2, c=BS, w=KEPT)
    av_recv = a2a_out.rearrange("j (b h r c w) -> b r j h c w", b=B, h=HS, r=2, c=BS, w=KEPT)
    # backward: chunk flat order (b, r, c, h, w) -- c-outer
    bv_send = bk_in.rearrange("j (b r c h w) -> j r c b h w", b=B, r=2, c=BS, h=HS, w=KEPT)
    bv_recv = bk_out.rearrange("j (b r c h w) -> b r j c h w", b=B, r=2, c=BS, h=HS, w=KEPT)

    def fwd_collective():
        tc_ref[0].strict_bb_all_engine_barrier()
        if USE_COLLECTIVES:
            nc.gpsimd.collective_compute(
                "AllToAll", ALU.bypass, replica_groups=RG,
                ins=[a2a_in[:].opt()], outs=[a2a_out[:].opt()])
        else:
            nc.gpsimd.dma_start(a2a_out[:], a2a_in[:])
        tc_ref[0].strict_bb_all_engine_barrier()

    def bwd_collective():
        tc_ref[0].strict_bb_all_engine_barrier()
        if USE_COLLECTIVES:
            nc.gpsimd.collective_compute(
                "AllToAll", ALU.bypass, replica_groups=RG,
                ins=[bk_in[:].opt()], outs=[bk_out[:].opt()])
        else:
            nc.gpsimd.dma_start(bk_out[:], bk_in[:])
        tc_ref[0].strict_bb_all_engine_barrier()

    tc_ref = [None]

    with tile.TileContext(nc) as tc:
        tc_ref[0] = tc
        with tc.tile_pool(name="const", bufs=1) as cpool, \
             tc.tile_pool(name="ps", bufs=6, space="PSUM") as pspool, \
             tc.tile_pool(name="ps2", bufs=2, space="PSUM") as pspool2:
            # ---- constants to SBUF (2D tiles, k-tiles on the free axis) ----
            dw_sb = cpool.tile([128, 2 * 130], BF16, tag="dw")
            cc_sb = cpool.tile([128, 6 * C], BF16, tag="cc")
            kpc_sb = cpool.tile([128, 6 * C], BF16, tag="kpc")
            kmc_sb = cpool.tile([128, 6 * C], BF16, tag="kmc")
            mm_sb = cpool.tile([128, 2 * 256], BF16, tag="mm")
            mp_sb = cpool.tile([128, 2 * 256], BF16, tag="mp")
            kc_sb = cpool.tile([128, 2 * 256], BF16, tag="kc")
            ksn_sb = cpool.tile([128, 2 * 256], BF16, tag="ksn")
            nc.gpsimd.dma_start(dw_sb[:], dw[:])
            nc.gpsimd.dma_start(mm_sb[:], mmp[:])
            nc.gpsimd.dma_start(mp_sb[:], mpp[:])
            nc.gpsimd.dma_start(kc_sb[:], kcp[:])
            nc.gpsimd.dma_start(ksn_sb[:], ksnp[:])
            nc.gpsimd.dma_start(cc_sb[:], cc[:])
            nc.gpsimd.dma_start(kpc_sb[:], kpc[:])
            nc.gpsimd.dma_start(kmc_sb[:], kmc[:])
            wm_sb = cpool.tile([KEPT, 2 * 128], BF16, tag="wm")
            wp_sb = cpool.tile([KEPT, 2 * 128], BF16, tag="wp")
            nc.gpsimd.dma_start(wm_sb[:], wmp[:])
            nc.gpsimd.dma_start(wp_sb[:], wpp[:])
            id_sb = cpool.tile([128, 128], BF16, tag="id")
            nc.gpsimd.dma_start(id_sb[:], identp[:])
            id32_sb = cpool.tile([128, 128], F32, tag="id32")
            nc.gpsimd.dma_start(id32_sb[:], identfp[:])
            w1p_sb = cpool.tile([BS, BS], BF16, tag="w1p")
            nc.gpsimd.dma_start(w1p_sb[:], w1p[:])
            w1m_sb = cpool.tile([BS, BS], BF16, tag="w1m")
            nc.gpsimd.dma_start(w1m_sb[:], w1m[:])
            w2p_sb = cpool.tile([BS, BS], BF16, tag="w2p")
            nc.gpsimd.dma_start(w2p_sb[:], w2p[:])
            w2m_sb = cpool.tile([BS, BS], BF16, tag="w2m")
            nc.gpsimd.dma_start(w2m_sb[:], w2m[:])
            b10_sb = cpool.tile([BS, 1], F32, tag="b10")
            nc.gpsimd.dma_start(b10_sb[:], b10[:])
            b11_sb = cpool.tile([BS, 1], F32, tag="b11")
            nc.gpsimd.dma_start(b11_sb[:], b11[:])
            b20_sb = cpool.tile([BS, 1], F32, tag="b20")
            nc.gpsimd.dma_start(b20_sb[:], b20[:])
            b21_sb = cpool.tile([BS, 1], F32, tag="b21")
            nc.gpsimd.dma_start(b21_sb[:], b21[:])
            bsv_sb = cpool.tile([BS, 1], F32, tag="bsv")
            nc.gpsimd.dma_start(bsv_sb[:], bsv[:])

            # ================= stage A =================
            with tc.tile_pool(name="phA", bufs=3) as pA, \
                 tc.tile_pool(name="phAx", bufs=32) as pAx:
                # hoist ALL x loads (gpsimd cast-DMAs) so the Pool queue never
                # stalls b=1 loads behind the b=0 collective
                xbf = {}
                for bb in range(8):
                    b = bb // 4
                    h0 = (bb % 4) * 4
                    for s in range(4):
                        h = h0 + s
                        for k in range(2):
                            t = pAx.tile([128, C], BF16, tag="xbf", name="xbf")
                            nc.gpsimd.dma_start(t[:], x[b, h, 128 * k:128 * (k + 1), :])
                            xbf[(b, h, k)] = t
                for bb in range(8):              # bundles: 4 slabs, same b
                    b = bb // 4
                    h0 = (bb % 4) * 4
                    stA = [pA.tile([128, 4 * 130], BF16, tag=f"stA{m}", name=f"stA{m}") for m in range(6)]
                    for s in range(4):
                        h = h0 + s
                        for m in range(6):
                            ps1 = pspool.tile([128, 130], F32, tag="ps")
                            for k in range(2):
                                nc.tensor.matmul(
                                    ps1[:],
                                    xbf[(b, h, k)][:, 128 * m:128 * (m + 1)],
                                    dw_sb[:, 130 * k:130 * (k + 1)],
                                    start=(k == 0), stop=(k == 1))
                            eng = nc.scalar if m % 2 else nc.vector
                            if eng is nc.scalar:
                                eng.activation(stA[m][:, s * 130:(s + 1) * 130], ps1[:], ACT_T.Copy)
                            else:
                                eng.tensor_copy(stA[m][:, s * 130:(s + 1) * 130], ps1[:])
                    # C-DFT for the bundle (Karatsuba: 18 chains instead of 24)
                    stv = [t[:].rearrange("p (s r) -> p s r", r=130) for t in stA]
                    stp = [pA.tile([128, 260], BF16, tag=f"stp{m}", name=f"stp{m}") for m in range(6)]
                    for k in range(6):
                        nc.vector.tensor_add(
                            stp[k][:].rearrange("p (s w) -> p s w", w=KEPT),
                            stv[k][:, :, 0:KEPT], stv[k][:, :, KEPT:130])
                    for m2 in range(6):
                        p1 = pspool.tile([128, 260], F32, tag="ps")
                        p2 = pspool.tile([128, 260], F32, tag="ps")
                        p3 = pspool.tile([128, 260], F32, tag="ps")
                        for k in range(6):
                            nc.tensor.matmul(p1[:], cc_sb[:, C * k + 128 * m2:C * k + 128 * (m2 + 1)],
                                             stp[k][:], start=(k == 0), stop=(k == 5))
                        for k in range(6):
                            nc.tensor.matmul(p2[:], kpc_sb[:, C * k + 128 * m2:C * k + 128 * (m2 + 1)],
                                             stv[k][:, :, 0:KEPT], start=(k == 0), stop=(k == 5))
                        for k in range(6):
                            nc.tensor.matmul(p3[:], kmc_sb[:, C * k + 128 * m2:C * k + 128 * (m2 + 1)],
                                             stv[k][:, :, KEPT:130], start=(k == 0), stop=(k == 5))
                        k1a = pA.tile([128, 260], F32, tag="k1a")
                        nc.scalar.activation(k1a[:], p1[:], ACT_T.Copy)
                        snr = pA.tile([128, 260], BF16, tag="snr")
                        sni = pA.tile([128, 260], BF16, tag="sni")
                        nc.vector.tensor_sub(snr[:], k1a[:], p3[:])
                        nc.vector.tensor_add(sni[:], k1a[:], p2[:])
                        # scatter to send buffer by destination core
                        c_lo = 128 * m2
                        c_hi = c_lo + 128
                        j = c_lo // BS
                        while 96 * j < c_hi:
                            lo = max(96 * j, c_lo)
                            hi = min(96 * j + 96, c_hi)
                            if hi > lo:
                                for reim, sn in ((0, snr), (1, sni)):
                                    nc.sync.dma_start(
                                        av_send[j, b, reim, lo - 96 * j:hi - 96 * j, h0:h0 + 4, :],
                                        sn[lo - c_lo:hi - c_lo, :].rearrange("p (s w) -> p s w", w=KEPT))
                            j += 1

            fwd_collective()

            # ================= stage B =================
            with tc.tile_pool(name="phB", bufs=1) as pB, \
                 tc.tile_pool(name="phBg", bufs=2) as pBg:
                reb = [pB.tile([128, BS * KEPT], BF16, tag=f"reb{b}", name=f"reb{b}") for b in range(B)]
                imb = [pB.tile([128, BS * KEPT], BF16, tag=f"imb{b}", name=f"imb{b}") for b in range(B)]
                for b in range(B):
                    for j in range(NCORES):
                        nc.sync.dma_start(
                            reb[b][16 * j:16 * (j + 1), :].rearrange("h (c w) -> h c w", c=BS),
                            av_recv[b, 0, j])
                        nc.sync.dma_start(
                            imb[b][16 * j:16 * (j + 1), :].rearrange("h (c w) -> h c w", c=BS),
                            av_recv[b, 1, j])
                rebv = [t[:].rearrange("p (c w) -> p c w", c=BS) for t in reb]
                imbv = [t[:].rearrange("p (c w) -> p c w", c=BS) for t in imb]
                s4r = pB.tile([BS, 256 * KEPT], BF16, tag="s4r")
                s4i = pB.tile([BS, 256 * KEPT], BF16, tag="s4i")
                s4rv = s4r[:].rearrange("p (n w) -> p n w", w=KEPT)
                s4iv = s4i[:].rearrange("p (n w) -> p n w", w=KEPT)
                s4rs = s4r[:].rearrange("p (b i hh w) -> p b i hh w", b=B, i=NCORES, w=KEPT)
                s4is = s4i[:].rearrange("p (b i hh w) -> p b i hh w", b=B, i=NCORES, w=KEPT)

                for gi, (w0, cnt) in enumerate(GROUPS):
                    toks = cnt * 256
                    a_ch = pBg.tile([BS, 1024], BF16, tag="a_ch")
                    an_ch = pBg.tile([BS, 1024], BF16, tag="an_ch")
                    nc.sync.dma_start(an_ch[:, 0:toks], an[:, w0 * 256:w0 * 256 + toks])
                    for wl in range(cnt):
                        wp_ = w0 + wl
                        psa = pspool.tile([BS, 256], F32, tag="ps")
                        nc.tensor.matmul(psa[:], rebv[0][:, :, wp_], mm_sb[:, 0:256], start=True, stop=False)
                        nc.tensor.matmul(psa[:], rebv[1][:, :, wp_], mm_sb[:, 256:512], start=False, stop=False)
                        nc.tensor.matmul(psa[:], imbv[0][:, :, wp_], mp_sb[:, 0:256], start=False, stop=False)
                        nc.tensor.matmul(psa[:], imbv[1][:, :, wp_], mp_sb[:, 256:512], start=False, stop=True)
                        nc.vector.tensor_copy(a_ch[:, wl * 256:(wl + 1) * 256], psa[:])
                    o1k = pBg.tile([BS, 1024], BF16, tag="o1k")
                    o1n = pBg.tile([BS, 1024], BF16, tag="o1n")
                    o2k = pBg.tile([BS, 1024], BF16, tag="o2k")
                    s_sb = pBg.tile([BS, 1024], BF16, tag="s_sb")
                    t1 = pBg.tile([BS, 1024], BF16, tag="t1")
                    t2 = pBg.tile([BS, 1024], F32, tag="t2")
                    nq = max(1, toks // 512)
                    qsz = toks // nq
                    for q in range(nq):
                        sl = slice(q * qsz, (q + 1) * qsz)
                        ps = pspool.tile([BS, 512], F32, tag="ps")
                        nc.tensor.matmul(ps[:, 0:qsz], w1p_sb[:], a_ch[:, sl], start=True, stop=False)
                        nc.tensor.matmul(ps[:, 0:qsz], w1m_sb[:], an_ch[:, sl], start=False, stop=True)
                        nc.scalar.activation(o1k[:, sl], ps[:, 0:qsz], ACT_T.Relu, bias=b10_sb[:])
                        ps = pspool.tile([BS, 512], F32, tag="ps")
                        nc.tensor.matmul(ps[:, 0:qsz], w1p_sb[:], an_ch[:, sl], start=True, stop=False)
                        nc.tensor.matmul(ps[:, 0:qsz], w1m_sb[:], a_ch[:, sl], start=False, stop=True)
                        nc.scalar.activation(o1n[:, sl], ps[:, 0:qsz], ACT_T.Relu, bias=b11_sb[:])
                        ps = pspool.tile([BS, 512], F32, tag="ps")
                        nc.tensor.matmul(ps[:, 0:qsz], w2p_sb[:], o1k[:, sl], start=True, stop=False)
                        nc.tensor.matmul(ps[:, 0:qsz], w2m_sb[:], o1n[:, sl], start=False, stop=True)
                        nc.scalar.activation(o2k[:, sl], ps[:, 0:qsz], ACT_T.Copy)
                        ps = pspool.tile([BS, 512], F32, tag="ps")
                        nc.tensor.matmul(ps[:, 0:qsz], w2p_sb[:], o1n[:, sl], start=True, stop=False)
                        nc.tensor.matmul(ps[:, 0:qsz], w2m_sb[:], o2k[:, sl], start=False, stop=True)
                        # s = (o2n + b21) + o2k ; softshrink: s - clamp(s, -l, l)
                        nc.vector.scalar_tensor_tensor(
                            s_sb[:, sl], ps[:, 0:qsz], b21_sb[:], o2k[:, sl],
                            op0=ALU.add, op1=ALU.add)
                        nc.vector.tensor_scalar(t1[:, sl], s_sb[:, sl], -0.01, 0.01,
                                                op0=ALU.max, op1=ALU.min)
                        nc.vector.tensor_sub(t2[:, sl], s_sb[:, sl], t1[:, sl])
                    # transpose s -> [m, c] and BH-DFT of s
                    s3 = [pBg.tile([128, 4 * BS], BF16, tag=f"s3_{hf}", name=f"s3_{hf}") for hf in range(2)]
                    for wl in range(cnt):
                        for hf in range(2):
                            pst = pspool2.tile([128, BS], F32, tag="pst")
                            nc.tensor.transpose(pst[:], t2[:, wl * 256 + hf * 128: wl * 256 + (hf + 1) * 128],
                                                id32_sb[:BS, :BS])
                            if hf:
                                nc.scalar.activation(s3[hf][:, wl * BS:(wl + 1) * BS], pst[:], ACT_T.Copy)
                            else:
                                nc.vector.tensor_copy(s3[hf][:, wl * BS:(wl + 1) * BS], pst[:])
                    for wl in range(cnt):
                        wp_ = w0 + wl
                        ps3r = pspool.tile([BS, 256], F32, tag="ps")
                        ps3i = pspool.tile([BS, 256], F32, tag="ps")
                        for hf in range(2):
                            nc.tensor.matmul(ps3r[:], s3[hf][:, wl * BS:(wl + 1) * BS],
                                             kc_sb[:, 256 * hf:256 * (hf + 1)],
                                             start=(hf == 0), stop=(hf == 1))
                        for hf in range(2):
                            nc.tensor.matmul(ps3i[:], s3[hf][:, wl * BS:(wl + 1) * BS],
                                             ksn_sb[:, 256 * hf:256 * (hf + 1)],
                                             start=(hf == 0), stop=(hf == 1))
                        if wp_ % 2:
                            nc.scalar.activation(s4rv[:, :, wp_], ps3r[:], ACT_T.Copy)
                            nc.vector.tensor_copy(s4iv[:, :, wp_], ps3i[:])
                        else:
                            nc.vector.tensor_copy(s4rv[:, :, wp_], ps3r[:])
                            nc.scalar.activation(s4iv[:, :, wp_], ps3i[:], ACT_T.Copy)
                # send back: bwd chunk layout [b][r][c][h][w]; s4 free = (b, i, hh, w)
                for i in range(NCORES):
                    nc.sync.dma_start(bv_send[i, 0], s4rs[:, :, i])
                    nc.sync.dma_start(bv_send[i, 1], s4is[:, :, i])

            bwd_collective()

            # ================= stage C =================
            with tc.tile_pool(name="phC", bufs=2) as pC, \
                 tc.tile_pool(name="phCx", bufs=6) as pCx:
                for b in range(B):
                    # bulk-load this batch's spectrum: [c-slab 128, (h16, w65)] per slab
                    rcr = [pC.tile([128, HS * KEPT], BF16, tag=f"rcr{k}", name=f"rcr{k}") for k in range(6)]
                    rci = [pC.tile([128, HS * KEPT], BF16, tag=f"rci{k}", name=f"rci{k}") for k in range(6)]
                    rcp = [pC.tile([128, HS * KEPT], BF16, tag=f"rcp{k}", name=f"rcp{k}") for k in range(6)]
                    for k in range(6):
                        for (j, lo, hi) in _slab_splits(k):
                            for reim, rc in ((0, rcr), (1, rci)):
                                nc.sync.dma_start(
                                    rc[k][lo - 128 * k:hi - 128 * k, :].rearrange(
                                        "p (h w) -> p h w", w=KEPT),
                                    bv_recv[b, reim, j, lo - BS * j:hi - BS * j, :, :])
                        nc.vector.tensor_add(rcp[k][:], rcr[k][:], rci[k][:])
                    rcrv = [t[:].rearrange("p (s w) -> p s w", w=KEPT) for t in rcr]
                    rciv = [t[:].rearrange("p (s w) -> p s w", w=KEPT) for t in rci]
                    rcpv = [t[:].rearrange("p (s w) -> p s w", w=KEPT) for t in rcp]
                    for h in range(HS):
                        re4 = pCx.tile([KEPT, C], BF16, tag="re4")
                        im4 = pCx.tile([KEPT, C], BF16, tag="im4")
                        for nch in range(2):
                            nsl = slice(384 * nch, 384 * (nch + 1))
                            p1 = pspool.tile([KEPT, 384], F32, tag="ps")
                            p2 = pspool.tile([KEPT, 384], F32, tag="ps")
                            p3 = pspool.tile([KEPT, 384], F32, tag="ps")
                            for k in range(6):
                                nc.tensor.matmul(p1[:], rcpv[k][:, h, :],
                                                 cc_sb[:, C * k + 384 * nch:C * k + 384 * (nch + 1)],
                                                 start=(k == 0), stop=(k == 5))
                            for k in range(6):
                                nc.tensor.matmul(p2[:], rcrv[k][:, h, :],
                                                 kpc_sb[:, C * k + 384 * nch:C * k + 384 * (nch + 1)],
                                                 start=(k == 0), stop=(k == 5))
                            for k in range(6):
                                nc.tensor.matmul(p3[:], rciv[k][:, h, :],
                                                 kmc_sb[:, C * k + 384 * nch:C * k + 384 * (nch + 1)],
                                                 start=(k == 0), stop=(k == 5))
                            k1s = pCx.tile([KEPT, 384], F32, tag="k1s")
                            nc.scalar.activation(k1s[:], p1[:], ACT_T.Copy)
                            nc.vector.tensor_sub(re4[:, nsl], k1s[:], p3[:])
                            nc.vector.tensor_add(im4[:, nsl], k1s[:], p2[:])
                        for wt in range(2):
                            xt = pCx.tile([128, C], F32, tag="xt2")
                            nc.sync.dma_start(xt[:], x[b, h, 128 * wt:128 * (wt + 1), :])
                            ot = pCx.tile([128, C], F32, tag="ot")
                            for nch in range(2):
                                nsl = slice(384 * nch, 384 * (nch + 1))
                                pso = pspool.tile([128, 384], F32, tag="ps")
                                nc.tensor.matmul(pso[:], wm_sb[:, 128 * wt:128 * (wt + 1)],
                                                 re4[:, nsl], start=True, stop=False)
                                nc.tensor.matmul(pso[:], wp_sb[:, 128 * wt:128 * (wt + 1)],
                                                 im4[:, nsl], start=False, stop=True)
                                nc.vector.tensor_add(ot[:, nsl], pso[:], xt[:, nsl])
                            nc.sync.dma_start(out[b, h, 128 * wt:128 * (wt + 1), :], ot[:])
    nc.finalize()
    return nc


_NC_CACHE = None
_LAST_IN_MAPS = None


def kernel(x, w1, b1, w2, b2):
    global _NC_CACHE
    x = np.ascontiguousarray(np.asarray(x, dtype=np.float32))
    w1 = np.asarray(w1, np.float32)
    b1 = np.asarray(b1, np.float32)
    w2 = np.asarray(w2, np.float32)
    b2 = np.asarray(b2, np.float32)

    consts = _host_consts()
    # an: flipped/rolled x, truncated to w' < KEPT, per-block, laid out [c, (w', m)]
    anf = np.roll(np.flip(x, axis=(1, 2)), shift=(1, 1), axis=(1, 2))[:, :, :KEPT, :]
    # (B, H, KEPT, C) -> per core j block: [c(96), w', b, h] -> (96, KEPT*256)
    in_maps = []
    for j in range(NCORES):
        blk = anf[:, :, :, j * BS:(j + 1) * BS]          # (2,128,65,96)
        an_j = blk.transpose(3, 2, 0, 1).reshape(BS, KEPT * B * H)
        m = {
            "x": np.ascontiguousarray(x[:, j * HS:(j + 1) * HS]),
            "an": np.ascontiguousarray(an_j).astype(ml_dtypes.bfloat16),
            "w1p": (0.5 * (w1[0, j] + w1[1, j])).astype(ml_dtypes.bfloat16),
            "w1m": (0.5 * (w1[0, j] - w1[1, j])).astype(ml_dtypes.bfloat16),
            "w2p": (0.5 * (w2[0, j] + w2[1, j])).astype(ml_dtypes.bfloat16),
            "w2m": (0.5 * (w2[0, j] - w2[1, j])).astype(ml_dtypes.bfloat16),
            "b10": np.ascontiguousarray(b1[0, j])[:, None],
            "b11": np.ascontiguousarray(b1[1, j])[:, None],
            "b20": np.ascontiguousarray(b2[0, j])[:, None],
            "b21": np.ascontiguousarray(b2[1, j])[:, None],
            "bsv": np.ascontiguousarray(
                b2[0, j] + b2[1, j]
                + (0.5 * (w2[0, j] - w2[1, j])).T.astype(np.float32) @ b2[0, j]
            )[:, None],
        }
        m.update(consts)
        in_maps.append(m)

    global _LAST_IN_MAPS
    _LAST_IN_MAPS = in_maps
    if _NC_CACHE is None:
        _NC_CACHE = build_nc()
    res = run_bass_kernel_spmd(_NC_CACHE, in_maps, core_ids=list(range(NCORES)))
    outs = [res.results[i]["out"] for i in range(NCORES)]
    return np.concatenate(outs, axis=1).astype(np.float32)


# revision 35
# speedup vs baseline: 1.0891x; 1.0891x over previous
"""AFNO2D (DHT-spectral block-MLP) Trainium2 kernel, 8-core SPMD.

Pipeline (per core):
  stage A (H-shard, 16 rows):  W-DFT 256->65 (trunc) then C-DFT 768, as matmuls
  AllToAll (split per batch, overlapped with stage A) -> C-shard
  stage B: fused (B,H)-DFT + Re+Im -> a ; block MLP + softshrink -> s ;
           (B,H)-DFT of s -> (re3,im3)
  AllToAll (split by w' halves, overlapped with stage B) -> H-shard
  stage C: C-IDFT (3-product Karatsuba), W-expand 65->256 + 1/numel, + x bias
All heavy compute is TensorE matmuls vs host-precomputed cos/sin DFT matrices.

Layouts: forward a2a chunk = [h][r][c][w] per batch (h-outer; stage-B reads
contiguous per h-row).  Backward a2a chunk = [b][r][c][h][w-half] (c-outer;
stage-B writes and stage-C reads get 1-4KB contiguous runs per partition).

Stage C Karatsuba: with A=cc.rcr, B=sc.rci, C2=cc.rci, D=sc.rcr:
  k1 = cc.(rcr+rci) = A+C2 ; k2 = -(cc+sc).rcr = -A-D ; k3 = (cc-sc).rci = C2-B
  re4 = k1-k3 = A+B ; im4 = k1+k2 = C2-D   (18 matmuls instead of 24)
"""
import math
import numpy as np
import ml_dtypes

import concourse.bass as bass
from concourse import bacc
import concourse.mybir as mybir
from concourse import tile
from concourse.bass_utils import run_bass_kernel_spmd

F32 = mybir.dt.float32
F32R = mybir.dt.float32r
BF16 = mybir.dt.bfloat16

B, H, W, C = 2, 128, 256, 768
NCORES = 8
HS = H // NCORES          # 16
KEPT = 65
BS = 96                   # block size (channels per core in stage B)
NB = 8
WSP = 32                  # backward a2a w' split point
CHUNKF = HS * 2 * BS * KEPT          # fwd chunk per batch: [h][r][c][w]
CHUNKB0 = B * 2 * BS * HS * WSP      # bwd chunk, w' 0:32
CHUNKB1 = B * 2 * BS * HS * (KEPT - WSP)   # bwd chunk, w' 32:65
ACT_T = mybir.ActivationFunctionType
ALU = mybir.AluOpType


def _cs(n, m=None):
    i = np.arange(n)[:, None].astype(np.float64)
    k = np.arange(m if m is not None else n)[None, :].astype(np.float64)
    th = 2.0 * np.pi * i * k / n
    return np.cos(th), np.sin(th)


def _host_consts():
    CW, SW = _cs(W, KEPT)
    CC, SC = _cs(C)
    CH, SH = _cs(H)
    F2 = np.array([[1.0, 1.0], [1.0, -1.0]])
    KBHc = np.kron(F2, CH)
    KBHs = np.kron(F2, SH)
    Mm = KBHc - KBHs
    Mp = KBHc + KBHs
    CW2, SW2 = _cs(W)
    numel = float(B * H * W * C)
    Wm = ((CW2 - SW2)[:KEPT] / numel)
    Wp = ((CW2 + SW2)[:KEPT] / numel)

    dw = np.zeros((W, 130), np.float64)
    dw[:, :KEPT] = CW
    dw[:, KEPT:130] = -SW

    def bf(a):
        return np.ascontiguousarray(a).astype(ml_dtypes.bfloat16)

    def kmaj(a, nk):          # (nk*128, F) -> (128, nk*F) k-tiles along free axis
        nkF = a.shape[1]
        return np.ascontiguousarray(
            a.reshape(nk, 128, nkF).transpose(1, 0, 2).reshape(128, nk * nkF))

    consts = {
        "dw": bf(kmaj(dw, 2)),
        "cc": bf(kmaj(CC, 6)),
        "sc": bf(kmaj(SC, 6)),
        "scn": bf(kmaj(-SC, 6)),
        "kpc": bf(kmaj(-(CC + SC), 6)),
        "kmc": bf(kmaj(CC - SC, 6)),
        "mm": bf(kmaj(Mm, 2)),
        "mp": bf(kmaj(Mp, 2)),
        "kc": bf(kmaj(KBHc, 2)),
        "ksn": bf(kmaj(-KBHs, 2)),
        "wm": bf(Wm.reshape(KEPT, 256)),
        "wp": bf(Wp.reshape(KEPT, 256)),
        "ident": bf(np.eye(128)),
        "identf": np.eye(128, dtype=np.float32),
    }
    return consts


# w'-groups for stage B: (start, count); 65 = 16*4 + 1
GROUPS = [(i * 4, 4) for i in range(16)] + [(64, 1)]
USE_COLLECTIVES = True

# stage C: c-slab [128k, 128k+128) split at 96-block boundaries -> (j, lo, hi)
def _slab_splits(k):
    out = []
    c_lo, c_hi = 128 * k, 128 * (k + 1)
    j = c_lo // BS
    while BS * j < c_hi:
        lo = max(BS * j, c_lo)
        hi = min(BS * j + BS, c_hi)
        if hi > lo:
            out.append((j, lo, hi))
        j += 1
    return out


def build_nc():
    nc = bacc.Bacc()
    x = nc.declare_dram_parameter("x", [B, HS, W, C], F32, isOutput=False)
    an = nc.declare_dram_parameter("an", [BS, KEPT * 256], BF16, isOutput=False)
    dw = nc.declare_dram_parameter("dw", [128, 2 * 130], BF16, isOutput=False)
    cc = nc.declare_dram_parameter("cc", [128, 6 * C], BF16, isOutput=False)
    sc = nc.declare_dram_parameter("sc", [128, 6 * C], BF16, isOutput=False)
    scn = nc.declare_dram_parameter("scn", [128, 6 * C], BF16, isOutput=False)
    kpc = nc.declare_dram_parameter("kpc", [128, 6 * C], BF16, isOutput=False)
    kmc = nc.declare_dram_parameter("kmc", [128, 6 * C], BF16, isOutput=False)
    mmp = nc.declare_dram_parameter("mm", [128, 2 * 256], BF16, isOutput=False)
    mpp = nc.declare_dram_parameter("mp", [128, 2 * 256], BF16, isOutput=False)
    kcp = nc.declare_dram_parameter("kc", [128, 2 * 256], BF16, isOutput=False)
    ksnp = nc.declare_dram_parameter("ksn", [128, 2 * 256], BF16, isOutput=False)
    wmp = nc.declare_dram_parameter("wm", [KEPT, 2 * 128], BF16, isOutput=False)
    wpp = nc.declare_dram_parameter("wp", [KEPT, 2 * 128], BF16, isOutput=False)
    identp = nc.declare_dram_parameter("ident", [128, 128], BF16, isOutput=False)
    identfp = nc.declare_dram_parameter("identf", [128, 128], F32, isOutput=False)
    w1p = nc.declare_dram_parameter("w1p", [BS, BS], BF16, isOutput=False)
    w1m = nc.declare_dram_parameter("w1m", [BS, BS], BF16, isOutput=False)
    w2p = nc.declare_dram_parameter("w2p", [BS, BS], BF16, isOutput=False)
    w2m = nc.declare_dram_parameter("w2m", [BS, BS], BF16, isOutput=False)
    b10 = nc.declare_dram_parameter("b10", [BS, 1], F32, isOutput=False)
    b11 = nc.declare_dram_parameter("b11", [BS, 1], F32, isOutput=False)
    b20 = nc.declare_dram_parameter("b20", [BS, 1], F32, isOutput=False)
    b21 = nc.declare_dram_parameter("b21", [BS, 1], F32, isOutput=False)
    out = nc.declare_dram_parameter("out", [B, HS, W, C], F32, isOutput=True)

    CHUNK = B * CHUNKF
    a2a_in = nc.dram_tensor("a2a_in", [NCORES, CHUNK], BF16)
    a2a_out = nc.dram_tensor("a2a_out", [NCORES, CHUNK], BF16)
    bk_in = nc.dram_tensor("bk_in", [NCORES, CHUNK], BF16)
    bk_out = nc.dram_tensor("bk_out", [NCORES, CHUNK], BF16)
    RG = [list(range(NCORES))]

    # DRAM views.  forward: chunk flat order (b, h, r, c, w)
    av_send = a2a_in.rearrange("j (b h r c w) -> j b r c h w", b=B, h=HS, r=2, c=BS, w=KEPT)
    av_recv = a2a_out.rearrange("j (b h r c w) -> b r j h c w", b=B, h=HS, r=2, c=BS, w=KEPT)
    # backward: chunk flat order (b, r, c, h, w) -- c-outer
    bv_send = bk_in.rearrange("j (b r c h w) -> j r c b h w", b=B, r=2, c=BS, h=HS, w=KEPT)
    bv_recv = bk_out.rearrange("j (b r c h w) -> b r j c h w", b=B, r=2, c=BS, h=HS, w=KEPT)

    def fwd_collective():
        tc_ref[0].strict_bb_all_engine_barrier()
        if USE_COLLECTIVES:
            nc.gpsimd.collective_compute(
                "AllToAll", ALU.bypass, replica_groups=RG,
                ins=[a2a_in[:].opt()], outs=[a2a_out[:].opt()])
        else:
            nc.gpsimd.dma_start(a2a_out[:], a2a_in[:])
        tc_ref[0].strict_bb_all_engine_barrier()

    def bwd_collective():
        tc_ref[0].strict_bb_all_engine_barrier()
        if USE_COLLECTIVES:
            nc.gpsimd.collective_compute(
                "AllToAll", ALU.bypass, replica_groups=RG,
                ins=[bk_in[:].opt()], outs=[bk_out[:].opt()])
        else:
            nc.gpsimd.dma_start(bk_out[:], bk_in[:])
        tc_ref[0].strict_bb_all_engine_barrier()

    tc_ref = [None]

    with tile.TileContext(nc) as tc:
        tc_ref[0] = tc
        with tc.tile_pool(name="const", bufs=1) as cpool, \
             tc.tile_pool(name="ps", bufs=6, space="PSUM") as pspool, \
             tc.tile_pool(name="ps2", bufs=2, space="PSUM") as pspool2:
            # ---- constants to SBUF (2D tiles, k-tiles on the free axis) ----
            dw_sb = cpool.tile([128, 2 * 130], BF16, tag="dw")
            cc_sb = cpool.tile([128, 6 * C], BF16, tag="cc")
            kpc_sb = cpool.tile([128, 6 * C], BF16, tag="kpc")
            kmc_sb = cpool.tile([128, 6 * C], BF16, tag="kmc")
            mm_sb = cpool.tile([128, 2 * 256], BF16, tag="mm")
            mp_sb = cpool.tile([128, 2 * 256], BF16, tag="mp")
            kc_sb = cpool.tile([128, 2 * 256], BF16, tag="kc")
            ksn_sb = cpool.tile([128, 2 * 256], BF16, tag="ksn")
            nc.gpsimd.dma_start(dw_sb[:], dw[:])
            nc.gpsimd.dma_start(mm_sb[:], mmp[:])
            nc.gpsimd.dma_start(mp_sb[:], mpp[:])
            nc.gpsimd.dma_start(kc_sb[:], kcp[:])
            nc.gpsimd.dma_start(ksn_sb[:], ksnp[:])
            nc.gpsimd.dma_start(cc_sb[:], cc[:])
            nc.gpsimd.dma_start(kpc_sb[:], kpc[:])
            nc.gpsimd.dma_start(kmc_sb[:], kmc[:])
            wm_sb = cpool.tile([KEPT, 2 * 128], BF16, tag="wm")
            wp_sb = cpool.tile([KEPT, 2 * 128], BF16, tag="wp")
            nc.gpsimd.dma_start(wm_sb[:], wmp[:])
            nc.gpsimd.dma_start(wp_sb[:], wpp[:])
            id_sb = cpool.tile([128, 128], BF16, tag="id")
            nc.gpsimd.dma_start(id_sb[:], identp[:])
            id32_sb = cpool.tile([128, 128], F32, tag="id32")
            nc.gpsimd.dma_start(id32_sb[:], identfp[:])
            w1p_sb = cpool.tile([BS, BS], BF16, tag="w1p")
            nc.gpsimd.dma_start(w1p_sb[:], w1p[:])
            w1m_sb = cpool.tile([BS, BS], BF16, tag="w1m")
            nc.gpsimd.dma_start(w1m_sb[:], w1m[:])
            w2p_sb = cpool.tile([BS, BS], BF16, tag="w2p")
            nc.gpsimd.dma_start(w2p_sb[:], w2p[:])
            w2m_sb = cpool.tile([BS, BS], BF16, tag="w2m")
            nc.gpsimd.dma_start(w2m_sb[:], w2m[:])
            b10_sb = cpool.tile([BS, 1], F32, tag="b10")
            nc.gpsimd.dma_start(b10_sb[:], b10[:])
            b11_sb = cpool.tile([BS, 1], F32, tag="b11")
            nc.gpsimd.dma_start(b11_sb[:], b11[:])
            b20_sb = cpool.tile([BS, 1], F32, tag="b20")
            nc.gpsimd.dma_start(b20_sb[:], b20[:])
            b21_sb = cpool.tile([BS, 1], F32, tag="b21")
            nc.gpsimd.dma_start(b21_sb[:], b21[:])

            # ================= stage A =================
            with tc.tile_pool(name="phA", bufs=2) as pA, \
                 tc.tile_pool(name="phAx", bufs=32) as pAx:
                # hoist ALL x loads (gpsimd cast-DMAs) so the Pool queue never
                # stalls b=1 loads behind the b=0 collective
                xbf = {}
                for bb in range(8):
                    b = bb // 4
                    h0 = (bb % 4) * 4
                    for s in range(4):
                        h = h0 + s
                        for k in range(2):
                            t = pAx.tile([128, C], BF16, tag="xbf", name="xbf")
                            nc.gpsimd.dma_start(t[:], x[b, h, 128 * k:128 * (k + 1), :])
                            xbf[(b, h, k)] = t
                for bb in range(8):              # bundles: 4 slabs, same b
                    b = bb // 4
                    h0 = (bb % 4) * 4
                    stA = [pA.tile([128, 4 * 130], BF16, tag=f"stA{m}", name=f"stA{m}") for m in range(6)]
                    for s in range(4):
                        h = h0 + s
                        for m in range(6):
                            ps1 = pspool.tile([128, 130], F32, tag="ps")
                            for k in range(2):
                                nc.tensor.matmul(
                                    ps1[:],
                                    xbf[(b, h, k)][:, 128 * m:128 * (m + 1)],
                                    dw_sb[:, 130 * k:130 * (k + 1)],
                                    start=(k == 0), stop=(k == 1))
                            eng = nc.scalar if m % 2 else nc.vector
                            if eng is nc.scalar:
                                eng.activation(stA[m][:, s * 130:(s + 1) * 130], ps1[:], ACT_T.Copy)
                            else:
                                eng.tensor_copy(stA[m][:, s * 130:(s + 1) * 130], ps1[:])
                    # C-DFT for the bundle (Karatsuba: 18 chains instead of 24)
                    stv = [t[:].rearrange("p (s r) -> p s r", r=130) for t in stA]
                    stp = [pA.tile([128, 260], BF16, tag=f"stp{m}", name=f"stp{m}") for m in range(6)]
                    for k in range(6):
                        nc.vector.tensor_add(
                            stp[k][:].rearrange("p (s w) -> p s w", w=KEPT),
                            stv[k][:, :, 0:KEPT], stv[k][:, :, KEPT:130])
                    for m2 in range(6):
                        p1 = pspool.tile([128, 260], F32, tag="ps")
                        p2 = pspool.tile([128, 260], F32, tag="ps")
                        p3 = pspool.tile([128, 260], F32, tag="ps")
                        for k in range(6):
                            nc.tensor.matmul(p1[:], cc_sb[:, C * k + 128 * m2:C * k + 128 * (m2 + 1)],
                                             stp[k][:], start=(k == 0), stop=(k == 5))
                        for k in range(6):
                            nc.tensor.matmul(p2[:], kpc_sb[:, C * k + 128 * m2:C * k + 128 * (m2 + 1)],
                                             stv[k][:, :, 0:KEPT], start=(k == 0), stop=(k == 5))
                        for k in range(6):
                            nc.tensor.matmul(p3[:], kmc_sb[:, C * k + 128 * m2:C * k + 128 * (m2 + 1)],
                                             stv[k][:, :, KEPT:130], start=(k == 0), stop=(k == 5))
                        k1a = pA.tile([128, 260], F32, tag="k1a")
                        nc.scalar.activation(k1a[:], p1[:], ACT_T.Copy)
                        snr = pA.tile([128, 260], BF16, tag="snr")
                        sni = pA.tile([128, 260], BF16, tag="sni")
                        nc.vector.tensor_sub(snr[:], k1a[:], p3[:])
                        nc.vector.tensor_add(sni[:], k1a[:], p2[:])
                        # scatter to send buffer by destination core
                        c_lo = 128 * m2
                        c_hi = c_lo + 128
                        j = c_lo // BS
                        while 96 * j < c_hi:
                            lo = max(96 * j, c_lo)
                            hi = min(96 * j + 96, c_hi)
                            if hi > lo:
                                for reim, sn in ((0, snr), (1, sni)):
                                    nc.sync.dma_start(
                                        av_send[j, b, reim, lo - 96 * j:hi - 96 * j, h0:h0 + 4, :],
                                        sn[lo - c_lo:hi - c_lo, :].rearrange("p (s w) -> p s w", w=KEPT))
                            j += 1

            fwd_collective()

            # ================= stage B =================
            with tc.tile_pool(name="phB", bufs=1) as pB, \
                 tc.tile_pool(name="phBg", bufs=3) as pBg:
                reb = [pB.tile([128, BS * KEPT], BF16, tag=f"reb{b}", name=f"reb{b}") for b in range(B)]
                imb = [pB.tile([128, BS * KEPT], BF16, tag=f"imb{b}", name=f"imb{b}") for b in range(B)]
                for b in range(B):
                    for j in range(NCORES):
                        nc.sync.dma_start(
                            reb[b][16 * j:16 * (j + 1), :].rearrange("h (c w) -> h c w", c=BS),
                            av_recv[b, 0, j])
                        nc.sync.dma_start(
                            imb[b][16 * j:16 * (j + 1), :].rearrange("h (c w) -> h c w", c=BS),
                            av_recv[b, 1, j])
                rebv = [t[:].rearrange("p (c w) -> p c w", c=BS) for t in reb]
                imbv = [t[:].rearrange("p (c w) -> p c w", c=BS) for t in imb]
                s4r = pB.tile([BS, 256 * KEPT], BF16, tag="s4r")
                s4i = pB.tile([BS, 256 * KEPT], BF16, tag="s4i")
                s4rv = s4r[:].rearrange("p (n w) -> p n w", w=KEPT)
                s4iv = s4i[:].rearrange("p (n w) -> p n w", w=KEPT)
                s4rs = s4r[:].rearrange("p (b i hh w) -> p b i hh w", b=B, i=NCORES, w=KEPT)
                s4is = s4i[:].rearrange("p (b i hh w) -> p b i hh w", b=B, i=NCORES, w=KEPT)

                for gi, (w0, cnt) in enumerate(GROUPS):
                    toks = cnt * 256
                    a_ch = pBg.tile([BS, 1024], BF16, tag="a_ch")
                    an_ch = pBg.tile([BS, 1024], BF16, tag="an_ch")
                    nc.sync.dma_start(an_ch[:, 0:toks], an[:, w0 * 256:w0 * 256 + toks])
                    for wl in range(cnt):
                        wp_ = w0 + wl
                        psa = pspool.tile([BS, 256], F32, tag="ps")
                        nc.tensor.matmul(psa[:], rebv[0][:, :, wp_], mm_sb[:, 0:256], start=True, stop=False)
                        nc.tensor.matmul(psa[:], rebv[1][:, :, wp_], mm_sb[:, 256:512], start=False, stop=False)
                        nc.tensor.matmul(psa[:], imbv[0][:, :, wp_], mp_sb[:, 0:256], start=False, stop=False)
                        nc.tensor.matmul(psa[:], imbv[1][:, :, wp_], mp_sb[:, 256:512], start=False, stop=True)
                        nc.vector.tensor_copy(a_ch[:, wl * 256:(wl + 1) * 256], psa[:])
                    o1k = pBg.tile([BS, 1024], BF16, tag="o1k")
                    o1n = pBg.tile([BS, 1024], BF16, tag="o1n")
                    o2k = pBg.tile([BS, 1024], BF16, tag="o2k")
                    s_sb = pBg.tile([BS, 1024], BF16, tag="s_sb")
                    t1 = pBg.tile([BS, 1024], BF16, tag="t1")
                    t2 = pBg.tile([BS, 1024], F32, tag="t2")
                    nq = max(1, toks // 512)
                    qsz = toks // nq
                    for q in range(nq):
                        sl = slice(q * qsz, (q + 1) * qsz)
                        ps = pspool.tile([BS, 512], F32, tag="ps")
                        nc.tensor.matmul(ps[:, 0:qsz], w1p_sb[:], a_ch[:, sl], start=True, stop=False)
                        nc.tensor.matmul(ps[:, 0:qsz], w1m_sb[:], an_ch[:, sl], start=False, stop=True)
                        nc.scalar.activation(o1k[:, sl], ps[:, 0:qsz], ACT_T.Relu, bias=b10_sb[:])
                        ps = pspool.tile([BS, 512], F32, tag="ps")
                        nc.tensor.matmul(ps[:, 0:qsz], w1p_sb[:], an_ch[:, sl], start=True, stop=False)
                        nc.tensor.matmul(ps[:, 0:qsz], w1m_sb[:], a_ch[:, sl], start=False, stop=True)
                        nc.scalar.activation(o1n[:, sl], ps[:, 0:qsz], ACT_T.Relu, bias=b11_sb[:])
                        ps = pspool.tile([BS, 512], F32, tag="ps")
                        nc.tensor.matmul(ps[:, 0:qsz], w2p_sb[:], o1k[:, sl], start=True, stop=False)
                        nc.tensor.matmul(ps[:, 0:qsz], w2m_sb[:], o1n[:, sl], start=False, stop=True)
                        nc.vector.tensor_scalar_add(o2k[:, sl], ps[:, 0:qsz], b20_sb[:])
                        ps = pspool.tile([BS, 512], F32, tag="ps")
                        nc.tensor.matmul(ps[:, 0:qsz], w2p_sb[:], o1n[:, sl], start=True, stop=False)
                        nc.tensor.matmul(ps[:, 0:qsz], w2m_sb[:], o2k[:, sl], start=False, stop=True)
                        # s = (o2n + b21) + o2k ; softshrink: s - clamp(s, -l, l)
                        nc.vector.scalar_tensor_tensor(
                            s_sb[:, sl], ps[:, 0:qsz], b21_sb[:], o2k[:, sl],
                            op0=ALU.add, op1=ALU.add)
                        nc.vector.tensor_scalar(t1[:, sl], s_sb[:, sl], -0.01, 0.01,
                                                op0=ALU.max, op1=ALU.min)
                        nc.vector.tensor_sub(t2[:, sl], s_sb[:, sl], t1[:, sl])
                    # transpose s -> [m, c] and BH-DFT of s
                    s3 = [pBg.tile([128, 4 * BS], BF16, tag=f"s3_{hf}", name=f"s3_{hf}") for hf in range(2)]
                    for wl in range(cnt):
                        for hf in range(2):
                            pst = pspool2.tile([128, BS], F32, tag="pst")
                            nc.tensor.transpose(pst[:], t2[:, wl * 256 + hf * 128: wl * 256 + (hf + 1) * 128],
                                                id32_sb[:BS, :BS])
                            if hf:
                                nc.scalar.activation(s3[hf][:, wl * BS:(wl + 1) * BS], pst[:], ACT_T.Copy)
                            else:
                                nc.vector.tensor_copy(s3[hf][:, wl * BS:(wl + 1) * BS], pst[:])
                    for wl in range(cnt):
                        wp_ = w0 + wl
                        ps3r = pspool.tile([BS, 256], F32, tag="ps")
                        ps3i = pspool.tile([BS, 256], F32, tag="ps")
                        for hf in range(2):
                            nc.tensor.matmul(ps3r[:], s3[hf][:, wl * BS:(wl + 1) * BS],
                                             kc_sb[:, 256 * hf:256 * (hf + 1)],
                                             start=(hf == 0), stop=(hf == 1))
                        for hf in range(2):
                            nc.tensor.matmul(ps3i[:], s3[hf][:, wl * BS:(wl + 1) * BS],
                                             ksn_sb[:, 256 * hf:256 * (hf + 1)],
                                             start=(hf == 0), stop=(hf == 1))
                        if wp_ % 2:
                            nc.scalar.activation(s4rv[:, :, wp_], ps3r[:], ACT_T.Copy)
                            nc.vector.tensor_copy(s4iv[:, :, wp_], ps3i[:])
                        else:
                            nc.vector.tensor_copy(s4rv[:, :, wp_], ps3r[:])
                            nc.scalar.activation(s4iv[:, :, wp_], ps3i[:], ACT_T.Copy)
                # send back: bwd chunk layout [b][r][c][h][w]; s4 free = (b, i, hh, w)
                for i in range(NCORES):
                    nc.sync.dma_start(bv_send[i, 0], s4rs[:, :, i])
                    nc.sync.dma_start(bv_send[i, 1], s4is[:, :, i])

            bwd_collective()

            # ================= stage C =================
            with tc.tile_pool(name="phC", bufs=2) as pC, \
                 tc.tile_pool(name="phCx", bufs=6) as pCx:
                for b in range(B):
                    # bulk-load this batch's spectrum: [c-slab 128, (h16, w65)] per slab
                    rcr = [pC.tile([128, HS * KEPT], BF16, tag=f"rcr{k}", name=f"rcr{k}") for k in range(6)]
                    rci = [pC.tile([128, HS * KEPT], BF16, tag=f"rci{k}", name=f"rci{k}") for k in range(6)]
                    rcp = [pC.tile([128, HS * KEPT], BF16, tag=f"rcp{k}", name=f"rcp{k}") for k in range(6)]
                    for k in range(6):
                        for (j, lo, hi) in _slab_splits(k):
                            for reim, rc in ((0, rcr), (1, rci)):
                                nc.sync.dma_start(
                                    rc[k][lo - 128 * k:hi - 128 * k, :].rearrange(
                                        "p (h w) -> p h w", w=KEPT),
                                    bv_recv[b, reim, j, lo - BS * j:hi - BS * j, :, :])
                        nc.vector.tensor_add(rcp[k][:], rcr[k][:], rci[k][:])
                    rcrv = [t[:].rearrange("p (s w) -> p s w", w=KEPT) for t in rcr]
                    rciv = [t[:].rearrange("p (s w) -> p s w", w=KEPT) for t in rci]
                    rcpv = [t[:].rearrange("p (s w) -> p s w", w=KEPT) for t in rcp]
                    for h in range(HS):
                        re4 = pCx.tile([KEPT, C], BF16, tag="re4")
                        im4 = pCx.tile([KEPT, C], BF16, tag="im4")
                        for nch in range(2):
                            nsl = slice(384 * nch, 384 * (nch + 1))
                            p1 = pspool.tile([KEPT, 384], F32, tag="ps")
                            p2 = pspool.tile([KEPT, 384], F32, tag="ps")
                            p3 = pspool.tile([KEPT, 384], F32, tag="ps")
                            for k in range(6):
                                nc.tensor.matmul(p1[:], rcpv[k][:, h, :],
                                                 cc_sb[:, C * k + 384 * nch:C * k + 384 * (nch + 1)],
                                                 start=(k == 0), stop=(k == 5))
                            for k in range(6):
                                nc.tensor.matmul(p2[:], rcrv[k][:, h, :],
                                                 kpc_sb[:, C * k + 384 * nch:C * k + 384 * (nch + 1)],
                                                 start=(k == 0), stop=(k == 5))
                            for k in range(6):
                                nc.tensor.matmul(p3[:], rciv[k][:, h, :],
                                                 kmc_sb[:, C * k + 384 * nch:C * k + 384 * (nch + 1)],
                                                 start=(k == 0), stop=(k == 5))
                            k1s = pCx.tile([KEPT, 384], F32, tag="k1s")
                            nc.scalar.activation(k1s[:], p1[:], ACT_T.Copy)
                            nc.vector.tensor_sub(re4[:, nsl], k1s[:], p3[:])
                            nc.vector.tensor_add(im4[:, nsl], k1s[:], p2[:])
                        for wt in range(2):
                            xt = pCx.tile([128, C], F32, tag="xt2")
                            nc.sync.dma_start(xt[:], x[b, h, 128 * wt:128 * (wt + 1), :])
                            ot = pCx.tile([128, C], F32, tag="ot")
                            for nch in range(2):
                                nsl = slice(384 * nch, 384 * (nch + 1))
                                pso = pspool.tile([128, 384], F32, tag="ps")
                                nc.tensor.matmul(pso[:], wm_sb[:, 128 * wt:128 * (wt + 1)],
                                                 re4[:, nsl], start=True, stop=False)
                                nc.tensor.matmul(pso[:], wp_sb[:, 128 * wt:128 * (wt + 1)],
                                                 im4[:, nsl], start=False, stop=True)
                                nc.vector.tensor_add(ot[:, nsl], pso[:], xt[:, nsl])
                            nc.sync.dma_start(out[b, h, 128 * wt:128 * (wt + 1), :], ot[:])
    nc.finalize()
    return nc


_NC_CACHE = None
_LAST_IN_MAPS = None


def kernel(x, w1, b1, w2, b2):
    global _NC_CACHE
    x = np.ascontiguousarray(np.asarray(x, dtype=np.float32))
    w1 = np.asarray(w1, np.float32)
    b1 = np.asarray(b1, np.float32)
    w2 = np.asarray(w2, np.float32)
    b2 = np.asarray(b2, np.float32)

    consts = _host_consts()
    # an: flipped/rolled x, truncated to w' < KEPT, per-block, laid out [c, (w', m)]
    anf = np.roll(np.flip(x, axis=(1, 2)), shift=(1, 1), axis=(1, 2))[:, :, :KEPT, :]
    # (B, H, KEPT, C) -> per core j block: [c(96), w', b, h] -> (96, KEPT*256)
    in_maps = []
    for j in range(NCORES):
        blk = anf[:, :, :, j * BS:(j + 1) * BS]          # (2,128,65,96)
        an_j = blk.transpose(3, 2, 0, 1).reshape(BS, KEPT * B * H)
        m = {
            "x": np.ascontiguousarray(x[:, j * HS:(j + 1) * HS]),
            "an": np.ascontiguousarray(an_j).astype(ml_dtypes.bfloat16),
            "w1p": (0.5 * (w1[0, j] + w1[1, j])).astype(ml_dtypes.bfloat16),
            "w1m": (0.5 * (w1[0, j] - w1[1, j])).astype(ml_dtypes.bfloat16),
            "w2p": (0.5 * (w2[0, j] + w2[1, j])).astype(ml_dtypes.bfloat16),
            "w2m": (0.5 * (w2[0, j] - w2[1, j])).astype(ml_dtypes.bfloat16),
            "b10": np.ascontiguousarray(b1[0, j])[:, None],
            "b11": np.ascontiguousarray(b1[1, j])[:, None],
            "b20": np.ascontiguousarray(b2[0, j])[:, None],
            "b21": np.ascontiguousarray(b2[1, j])[:, None],
        }
        m.update(consts)
        in_maps.append(m)

    global _LAST_IN_MAPS
    _LAST_IN_MAPS = in_maps
    if _NC_CACHE is None:
        _NC_CACHE = build_nc()
    res = run_bass_kernel_spmd(_NC_CACHE, in_maps, core_ids=list(range(NCORES)))
    outs = [res.results[i]["out"] for i in range(NCORES)]
    return np.concatenate(outs, axis=1).astype(np.float32)
